# revision 24
# baseline (speedup 1.0000x reference)
"""Trainium2 Bass kernel for nn_MultiHeadAttention_34162169872901.

MultiHeadAttention (B=4, S=2048, d_model=512, 8 heads, d_k=64) with a
relative-position bias table (511 entries, clamp +-255) and an all-ones mask.

Sharding (8 NeuronCores): core c handles batch b = c//2 and 4 of the 8 heads
(c%2 selects the head half) -- data parallel on B, tensor parallel on heads.
Each core computes its 4 heads' Q/K/V projections, the full attention for its
batch, normalization, and its partial output projection; the host sums the two
partial outputs per batch (and adds the output bias bo).

Layout: scores are computed transposed (S^T[k, q], k on partitions); the
relative-position bias becomes per-(k-tile, q-chunk) Toeplitz blocks
(out-of-band blocks fold into the exp's per-partition bias; in-band blocks
multiply exp(scores) by precomputed exp(bias) blocks on DVE).  The AV matmul
uses V plus a ones column as the stationary operand, producing ctx^T[d, q]
and the softmax denominator in one accumulation chain.  The O-projection
stacks two heads per matmul (contraction 128).

The whole kernel is one software pipeline: body i issues scores(i) [PE],
exp(i-1) [ACT], bias-mul(i-1) [DVE], AV(i-2) [PE].  The hp1 K/Q projections
and the V projection run as pseudo-units interleaved into the first bodies,
so the scalar engine's ~145us of exp work starts ~12us into the kernel.
Normalization and the O-projection are deferred into later bodies, 1-2 ops
per body.  DMA issue instructions cost ~0.6us of the issuing engine's time,
so nothing DMA-related sits on the scalar (ACT) queue once exp begins.
"""

import sys
import types
from collections import defaultdict

import numpy as np

B = 4
S = 2048
D = 512
NHEAD = 8
DK = 64
NCORES = 8
MAX_REL = 255
NKT = S // 128   # 16 k-tiles
NU = S // 512    # 4 q-units


def _install_axon_hooks():
    """Provide antenv.axon_hooks (missing in this image) so bass_utils'
    trace path can be used; harmless when tracing is off."""
    try:
        import antenv
    except ImportError:
        return
    try:
        from antenv.axon_hooks import get_axon_ntff_profile_hook  # noqa: F401
        return
    except ImportError:
        pass
    hook = None
    try:
        from trn_agent_boot.trn_boot import _ntff_profile_via_ctypes
        hook = _ntff_profile_via_ctypes("/opt/axon/libaxon_pjrt.so")
    except Exception:
        hook = None
    m = types.ModuleType("antenv.axon_hooks")
    m.get_axon_ntff_profile_hook = lambda: hook
    m.set_axon_ntff_profile_hook = lambda h: None
    sys.modules["antenv.axon_hooks"] = m
    antenv.axon_hooks = m


_install_axon_hooks()

import concourse.bass as bass  # noqa: E402
import concourse.bacc as bacc  # noqa: E402
import concourse.mybir as mybir  # noqa: E402
from concourse import tile  # noqa: E402
from concourse.bass_utils import run_bass_kernel_spmd  # noqa: E402
from concourse.vector_clock import ScopedClock as _ScopedClock  # noqa: E402

f32 = mybir.dt.float32
f32r = mybir.dt.float32r
bf16 = mybir.dt.bfloat16
f16 = mybir.dt.float16
AF = mybir.ActivationFunctionType


def _patched_drain_and_barrier(self, tick_clock, wait_clock):
    # walrus in this container rejects >2 sem waits on one instruction; emit
    # the tail-drain waits as standalone wait instructions instead.
    nc = self.nc
    dummy = mybir.InstNoOp(name="drain-wait-probe", engine=mybir.EngineType.SP)
    wait_clock.add_sem_waits(dummy, _ScopedClock({None: tick_clock.global_clock}))
    handles = {h.name: h for h in self.sems.allocated().values()}
    si = dummy.sync_info
    for w in (si.on_wait if si is not None else []):
        nc.sync.wait_ge(handles[w.ant_name], w.wait_value)
    nc.sync.drain()
    nc.all_engine_barrier()
    popped = nc._tile_sem_poison_stack.pop()
    assert popped is self._sem_poison
    nc.clear_and_free_semaphores(list(self.sems.allocated().values()))
    nc.all_engine_barrier()


tile.TileContext._drain_and_barrier = _patched_drain_and_barrier


def _delta(t, u):
    # key-tile offset minus query-chunk offset; bias entry index is
    # delta + (p - f) + 255 clipped to [0, 510]
    return 128 * t - 512 * u


def _cls(t, u):
    d = _delta(t, u)
    if d <= -384:
        return 1  # whole block clamps to table[0]
    if d >= 768:
        return 2  # whole block clamps to table[510]
    return 0      # in-band: needs the Toeplitz block


def _didx(t, u):
    return (_delta(t, u) + 256) // 128  # 0..7 for in-band blocks


def _unit_list():
    """Pipeline unit sequence: attention units for (u, hp, g, ah) with ALL
    projections interleaved as pseudo units (kq: K/Q projection chunk,
    vg: V projection pair) so there is no serial projection phase at all.
    kq `which`: 0=K hp0, 1=Q hp0, 2=K hp1, 3=Q hp1."""
    units = [("kq", 0, 0, 0), ("kq", 1, 0, 0), ("vg", 0, 0, 0)]
    after_hp0 = {0: [("vg", 1, 0, 0)],
                 1: [("kq", 0, 1, 0), ("vg", 2, 0, 0)],
                 2: [("vg", 3, 0, 0)],
                 3: [("kq", 0, 2, 0), ("vg", 4, 0, 0)],
                 4: [("vg", 5, 0, 0), ("kq", 2, 0, 0)],
                 5: [("kq", 0, 3, 0), ("vg", 6, 0, 0)],
                 6: [("vg", 7, 0, 0), ("kq", 3, 0, 0)]}
    for g in range(8):
        units.append(("att", 0, 0, (g, 0)))
        units.append(("att", 0, 0, (g, 1)))
        units.extend(after_hp0.get(g, []))
    after_hp1 = {0: [("kq", 2, 1, 0)],
                 2: [("kq", 2, 2, 0), ("kq", 1, 1, 0)],
                 4: [("kq", 2, 3, 0), ("kq", 1, 2, 0)],
                 6: [("kq", 1, 3, 0), ("kq", 3, 1, 0)],
                 7: [("kq", 3, 2, 0), ("kq", 3, 3, 0)]}
    for g in range(8):
        units.append(("att", 0, 1, (g, 0)))
        units.append(("att", 0, 1, (g, 1)))
        units.extend(after_hp1.get(g, []))
    for u in range(NU):
        for hp in range(2):
            if u == 0:
                continue
            for g in range(8):
                for ah in range(2):
                    units.append(("att", u, hp, (g, ah)))
    return units


def build_program():
    nc = bacc.Bacc()

    xqT = nc.declare_dram_parameter("xqT", [D, S], f16, isOutput=False)
    xkT = nc.declare_dram_parameter("xkT", [D, S], f16, isOutput=False)
    xvT = nc.declare_dram_parameter("xvT", [D, S], f16, isOutput=False)
    wq = nc.declare_dram_parameter("wq", [128, 4, 256], f16, isOutput=False)
    wk = nc.declare_dram_parameter("wk", [128, 4, 256], f16, isOutput=False)
    wv = nc.declare_dram_parameter("wv", [128, 4, 256], f16, isOutput=False)
    wo = nc.declare_dram_parameter("wo", [128, 2, 512], f16, isOutput=False)
    ebd = nc.declare_dram_parameter("eb", [128, 4, 8, 512], f16, isOutput=False)
    cbd = nc.declare_dram_parameter("cb", [128, 4, 3], f32, isOutput=False)
    bcd = nc.declare_dram_parameter("bcp", [2, 128], f16, isOutput=False)
    outd = nc.declare_dram_parameter("out", [S, D], f32, isOutput=True)

    with tile.TileContext(nc) as tc:
        with (
            tc.tile_pool(name="sb", bufs=1) as pool,
            tc.tile_pool(name="wkk", bufs=4) as wkp,
            tc.tile_pool(name="ps", bufs=1, space="PSUM") as psp,
        ):
            # ---- persistent SBUF tiles -------------------------------------
            wq_sb = pool.tile([128, 4, 256], f16, tag="wq", name="wq_sb")
            wk_sb = pool.tile([128, 4, 256], f16, tag="wk", name="wk_sb")
            wv_sb = pool.tile([128, 4, 256], f16, tag="wv", name="wv_sb")
            wo_sb = pool.tile([128, 2, 512], f16, tag="wo", name="wo_sb")
            eb_sb = pool.tile([128, 4, 8, 512], f16, tag="eb", name="eb_sb")
            cb_sb = pool.tile([128, 4, 3], f32, tag="cb", name="cb_sb")
            qt_sb = pool.tile([128, 2, S], f16, tag="qt", name="qt_sb")
            kt_sb = pool.tile([128, 2, S], f16, tag="kt", name="kt_sb")
            v_sb = pool.tile([128, NKT, 4 * 65], f16, tag="v", name="v_sb")
            xq_sb = pool.tile([128, 4, S], f16, tag="xq", name="xq_sb")
            xk_sb = pool.tile([128, 4, S], f16, tag="xk", name="xk_sb")
            xv_sb = pool.tile([128, 4, S], f16, tag="xv", name="xv_sb")
            bc1 = pool.tile([2, 128], f16, tag="bc1", name="bc1")
            warm = pool.tile([128, 16], f32, tag="warm", name="warm")

            # ---- input DMAs ------------------------------------------------
            nc.sync.dma_start(wk_sb[:], wk[:])
            nc.scalar.dma_start(wq_sb[:], wq[:])
            nc.gpsimd.dma_start(cb_sb[:], cbd[:])
            nc.gpsimd.dma_start(bc1[:], bcd[:])
            nc.gpsimd.dma_start(wv_sb[:], wv[:])
            for sc in range(4):
                cols = slice(sc * 512, (sc + 1) * 512)
                nc.sync.dma_start(
                    xk_sb[:, :, cols],
                    xkT[:, cols].rearrange("(c p) x -> p c x", p=128))
                nc.scalar.dma_start(
                    xq_sb[:, :, cols],
                    xqT[:, cols].rearrange("(c p) x -> p c x", p=128))
                nc.gpsimd.dma_start(
                    xv_sb[:, :, cols],
                    xvT[:, cols].rearrange("(c p) x -> p c x", p=128))
            # eb is big (4.2MB): hp0's half is needed by ~body 3, hp1's much
            # later; keep both on the otherwise-idle gpsimd queue.
            nc.gpsimd.dma_start(eb_sb[:, 0:2, :, :], ebd[:, 0:2, :, :])
            nc.gpsimd.dma_start(wo_sb[:], wo[:])
            nc.gpsimd.dma_start(eb_sb[:, 2:4, :, :], ebd[:, 2:4, :, :])

            # preload the exp table while DMAs stream in
            nc.vector.memset(warm[:], 0.0)
            nc.scalar.activation(warm[:], warm[:], AF.Exp, bias=0.0, scale=1.0)
            nc.vector.memset(
                v_sb.rearrange("p s (h x) -> p s h x", x=65)[:, :, :, 64:65],
                1.0)
            # PE clock warm-up: dummy matmuls while input DMAs land, so the
            # first real projections don't run at the cold p-state
            dum = pool.tile([128, 512], f16, tag="dum", name="dum")
            nc.vector.memset(dum[:], 0.0)
            podum = psp.tile([128, 512], f32, tag="po", bufs=1, name="podum")
            for j in range(8):
                nc.tensor.matmul(podum[:], lhsT=dum[:, 0:128], rhs=dum[:],
                                 start=True, stop=True)

            # ---- the pipeline ----------------------------------------------
            UNITS = _unit_list()
            NUNITS = len(UNITS)

            state = {}
            ctxp_tiles = {}
            ctxf_tiles = {}
            cx_tiles = {}
            navs = {}
            attpos = defaultdict(int)
            sched = defaultdict(list)

            def defer(body, fn):
                sched[body].append(fn)

            def issue_S(i):
                kind, a1, a2, a3 = UNITS[i]
                sct = psp.tile([128, 1024], f32, tag="sct", bufs=2,
                               name=f"sct{i}")
                if kind == "att":
                    u, hp, (g, ah) = a1, a2, a3
                    for ti in range(2):
                        t = 2 * g + ti
                        nc.tensor.matmul(
                            sct[:, ti * 512:(ti + 1) * 512],
                            lhsT=kt_sb[ah * 64:(ah + 1) * 64, hp,
                                       t * 128:(t + 1) * 128],
                            rhs=qt_sb[ah * 64:(ah + 1) * 64, hp,
                                      u * 512:(u + 1) * 512],
                            start=True, stop=True,
                            tile_position=(ah * 64, 0),
                        )
                elif kind == "vg":
                    g = a1
                    for s2 in range(2):
                        st = 2 * g + s2
                        for ct in range(4):
                            nc.tensor.matmul(
                                sct[:, s2 * 256:(s2 + 1) * 256],
                                lhsT=xv_sb[:, ct, st * 128:(st + 1) * 128],
                                rhs=wv_sb[:, ct, :],
                                start=(ct == 0), stop=(ct == 3),
                            )
                else:  # kq: K/Q projection, one sc chunk
                    which, sc = a1, a2
                    w_t = wq_sb if which % 2 else wk_sb
                    x_t = xq_sb if which % 2 else xk_sb
                    hp = which // 2
                    for ct in range(4):
                        nc.tensor.matmul(
                            sct[:, 0:512],
                            lhsT=w_t[:, ct, hp * 128:(hp + 1) * 128],
                            rhs=x_t[:, ct, sc * 512:(sc + 1) * 512],
                            start=(ct == 0), stop=(ct == 3),
                        )
                state[i] = {"sct": sct}

            def issue_EM(i):
                kind, a1, a2, a3 = UNITS[i]
                sct = state[i]["sct"]
                if kind == "att":
                    u, hp, (g, ah) = a1, a2, a3
                    lh = 2 * hp + ah
                    cls = _cls(2 * g, u)
                    pt = wkp.tile([128, 1024], f16, tag="pt", bufs=4,
                                  name=f"pt{i}")
                    nc.scalar.activation(
                        pt[:], sct[:], AF.Exp,
                        bias=cb_sb[:, lh, cls:cls + 1], scale=1.0,
                    )
                    if cls == 0:
                        src = wkp.tile([128, 1024], f16, tag="srcx", bufs=4,
                                       name=f"src{i}")
                        for ti in range(2):
                            nc.vector.tensor_mul(
                                src[:, ti * 512:(ti + 1) * 512],
                                pt[:, ti * 512:(ti + 1) * 512],
                                eb_sb[:, lh, _didx(2 * g + ti, u), :],
                            )
                    else:
                        src = pt
                    state[i]["src"] = src
                elif kind == "vg":
                    g = a1
                    nc.vector.tensor_copy(
                        v_sb[:, 2 * g:2 * g + 2, :]
                        .rearrange("p s (h x) -> p s h x", x=65)[:, :, :, 0:64],
                        sct[:, 0:512].rearrange("p (s h x) -> p s h x",
                                                s=2, h=4),
                    )
                else:
                    which, sc = a1, a2
                    dst = qt_sb if which % 2 else kt_sb
                    nc.vector.tensor_copy(
                        dst[:, which // 2, sc * 512:(sc + 1) * 512],
                        sct[:, 0:512])

            def issue_A(i):
                _, u, hp, (g, ah) = UNITS[i]
                lh = 2 * hp + ah
                key = (u, hp, ah)
                if key not in ctxp_tiles:
                    ctxp_tiles[key] = psp.tile(
                        [65, 512], f32, tag="ctx", bufs=2,
                        name=f"ctx{u}{hp}{ah}")
                    navs[key] = 0
                src = state[i]["src"]
                for ti in range(2):
                    t = 2 * g + ti
                    navs[key] += 1
                    nc.tensor.matmul(
                        ctxp_tiles[key][:],
                        lhsT=v_sb[:, t, lh * 65:(lh + 1) * 65],
                        rhs=src[:, ti * 512:(ti + 1) * 512],
                        start=(navs[key] == 1), stop=(navs[key] == 16),
                    )

            def sched_norm(i_last, u, hp):
                # A(i_last) issues at body i_last+2; normalization follows.
                # The very last block gets a dense schedule (its deferred
                # bodies all fire back-to-back in the pipeline tail, and the
                # ACT queue is idle there, so its issue slots are free).
                b0 = i_last + 3
                last = (u == NU - 1 and hp == 1)

                def cp(ah):
                    def fn():
                        ctxf = wkp.tile([65, 512], f32, tag="cxf", bufs=4,
                                        name=f"cxf{u}{hp}{ah}")
                        nc.vector.tensor_copy(ctxf[:],
                                              ctxp_tiles[(u, hp, ah)][:])
                        ctxf_tiles[(u, hp, ah)] = ctxf
                    return fn

                defer(b0, cp(0))
                defer(b0 if last else b0 + 1, cp(1))

                lpt = {}

                def lp_dma():
                    lp = wkp.tile([2, 512], f32, tag="lp", bufs=2,
                                  name=f"lp{u}{hp}")
                    q0 = nc.sync
                    q1 = nc.scalar if last else nc.sync
                    q0.dma_start(lp[0:1, :],
                                 ctxf_tiles[(u, hp, 0)][64:65, :])
                    q1.dma_start(lp[1:2, :],
                                 ctxf_tiles[(u, hp, 1)][64:65, :])
                    lpt["lp"] = lp
                defer(b0 + 1, lp_dma)

                def recip():
                    linv = wkp.tile([2, 512], f32, tag="linv", bufs=2,
                                    name=f"linv{u}{hp}")
                    linvb = wkp.tile([2, 512], f16, tag="linvb", bufs=2,
                                     name=f"linvb{u}{hp}")
                    nc.vector.reciprocal_approx_fast(linv[:], lpt["lp"][:])
                    nc.vector.tensor_scalar_mul(linvb[:], linv[:], 16.0)
                    lpt["linvb"] = linvb
                defer(b0 + 2 if last else b0 + 3, recip)

                def bc_mm():
                    bc = psp.tile([128, 512], f32, tag="bc", bufs=1,
                                  name=f"bc{u}{hp}")
                    nc.tensor.matmul(bc[:], lhsT=bc1[:], rhs=lpt["linvb"][:],
                                     start=True, stop=True)
                    lpt["bc"] = bc
                defer(b0 + 3 if last else b0 + 4, bc_mm)

                def cx_mul(ah):
                    def fn():
                        key = (u, hp)
                        if key not in cx_tiles:
                            cx_tiles[key] = wkp.tile(
                                [128, 512], f16, tag="cx", bufs=4,
                                name=f"cx{u}{hp}")
                        nc.vector.tensor_mul(
                            cx_tiles[key][ah * 64:(ah + 1) * 64, :],
                            lpt["bc"][ah * 64:(ah + 1) * 64, :],
                            ctxf_tiles[(u, hp, ah)][0:64, :],
                        )
                    return fn
                cb_ = b0 + 4 if last else b0 + 5
                defer(cb_, cx_mul(0))
                defer(cb_, cx_mul(1))

                if hp == 1:
                    pb = b0 + 5 if last else b0 + 7
                    step = 2 if last else 3
                    for qs in range(4):
                        pot = {}

                        def po_mm(qs, pot):
                            def fn():
                                pot["po"] = psp.tile(
                                    [128, 512], f32, tag="po", bufs=1,
                                    name=f"po{u}{qs}")
                                for hp2 in range(2):
                                    nc.tensor.matmul(
                                        pot["po"][:],
                                        lhsT=cx_tiles[(u, hp2)][
                                            :, qs * 128:(qs + 1) * 128],
                                        rhs=wo_sb[:, hp2, :],
                                        start=(hp2 == 0), stop=(hp2 == 1),
                                    )
                            return fn

                        def po_out(qs, pot):
                            def fn():
                                ob = wkp.tile([128, 512], f32, tag="ob",
                                              bufs=2, name=f"ob{u}{qs}")
                                nc.vector.tensor_copy(ob[:], pot["po"][:])
                                qo = nc.gpsimd if (last and qs % 2) else nc.sync
                                qo.dma_start(
                                    outd[u * 512 + qs * 128:
                                         u * 512 + (qs + 1) * 128, :],
                                    ob[:],
                                )
                            return fn

                        defer(pb + qs * step, po_mm(qs, pot))
                        defer(pb + qs * step + 1, po_out(qs, pot))

            # ---- pipeline loop ---------------------------------------------
            for body in range(NUNITS + 64):
                if body < NUNITS:
                    issue_S(body)
                    kind, u, hp, a3 = UNITS[body]
                    if kind == "att":
                        g, ah = a3
                        p = attpos[(u, hp)]
                        attpos[(u, hp)] += 1
                        skew = 4 if p < 2 else 2
                        defer(body + skew, (lambda i: lambda: issue_A(i))(body))
                        if g == 7 and ah == 1:
                            sched_norm(body, u, hp)
                if 0 <= body - 1 < NUNITS:
                    issue_EM(body - 1)
                for fn in sched.pop(body, []):
                    fn()

    nc.compile()
    return nc


_PROGRAM = None


def _get_program():
    global _PROGRAM
    if _PROGRAM is None:
        _PROGRAM = build_program()
    return _PROGRAM


# index table for the in-band Toeplitz bias blocks, shared across heads
_IDX = None


def _idx_table():
    global _IDX
    if _IDX is None:
        p = np.arange(128)[:, None]
        f = np.arange(512)[None, :]
        blocks = []
        for didx in range(8):
            delta = didx * 128 - 256
            blocks.append(np.clip(delta + p - f + 255, 0, 510))
        _IDX = np.stack(blocks, axis=0)  # [8, 128, 512]
    return _IDX


def kernel(**inputs):
    query = np.asarray(inputs["query"], dtype=np.float32)
    key = np.asarray(inputs["key"], dtype=np.float32)
    value = np.asarray(inputs["value"], dtype=np.float32)
    mask = np.asarray(inputs["mask"])
    Wq = np.asarray(inputs["Wq"], dtype=np.float32)
    Wk = np.asarray(inputs["Wk"], dtype=np.float32)
    Wv = np.asarray(inputs["Wv"], dtype=np.float32)
    Wo = np.asarray(inputs["Wo"], dtype=np.float32)
    bo = np.asarray(inputs["bo"], dtype=np.float32)
    rel_bias = np.asarray(inputs["rel_bias"], dtype=np.float32)

    if not np.all(mask != 0):
        raise NotImplementedError("kernel assumes an all-ones attention mask")

    nc = _get_program()
    idx = _idx_table()
    scale = np.float32(1.0 / np.sqrt(DK))

    in_maps = []
    for c in range(NCORES):
        b = c // 2
        hbase = (c % 2) * 4
        rows = slice(hbase * 64, (hbase + 4) * 64)

        wq_arr = np.ascontiguousarray(
            (Wq[rows, :] * scale).T.reshape(4, 128, 256).swapaxes(0, 1))
        wk_arr = np.ascontiguousarray(
            Wk[rows, :].T.reshape(4, 128, 256).swapaxes(0, 1))
        wv_arr = np.ascontiguousarray(
            Wv[rows, :].T.reshape(4, 128, 256).swapaxes(0, 1))

        wo_arr = np.empty((128, 2, 512), dtype=np.float32)
        eb_arr = np.empty((128, 4, 8, 512), dtype=np.float16)
        cb_arr = np.zeros((128, 4, 3), dtype=np.float32)
        for lh in range(4):
            g = hbase + lh
            wo_arr[(lh % 2) * 64:(lh % 2) * 64 + 64, lh // 2, :] = \
                Wo[:, g * 64:(g + 1) * 64].T * (1.0 / 16.0)
            tbl = rel_bias[g]
            eb_arr[:, lh, :, :] = np.exp(tbl)[idx].transpose(1, 0, 2)
            cb_arr[:, lh, 1] = tbl[0]
            cb_arr[:, lh, 2] = tbl[510]

        bcp = np.zeros((2, 128), dtype=np.float16)
        bcp[0, 0:64] = 1.0
        bcp[1, 64:128] = 1.0

        bf = np.float16
        in_maps.append({
            "bcp": bcp,
            "xqT": np.ascontiguousarray(query[b].T).astype(bf),
            "xkT": np.ascontiguousarray(key[b].T).astype(bf),
            "xvT": np.ascontiguousarray(value[b].T).astype(bf),
            "wq": wq_arr.astype(bf), "wk": wk_arr.astype(bf),
            "wv": wv_arr.astype(bf), "wo": wo_arr.astype(bf),
            "eb": eb_arr, "cb": cb_arr,
        })

    res = run_bass_kernel_spmd(nc, in_maps, list(range(NCORES)), trace=False)

    out = np.zeros((B, S, D), dtype=np.float32)
    for c in range(NCORES):
        out[c // 2] += res.results[c]["out"]
    out += bo[None, None, :]
    return out


# revision 25
# speedup vs baseline: 1.0147x; 1.0147x over previous
"""Trainium2 Bass kernel for nn_MultiHeadAttention_34162169872901.

MultiHeadAttention (B=4, S=2048, d_model=512, 8 heads, d_k=64) with a
relative-position bias table (511 entries, clamp +-255) and an all-ones mask.

Sharding (8 NeuronCores): core c handles batch b = c//2 and 4 of the 8 heads
(c%2 selects the head half) -- data parallel on B, tensor parallel on heads.
Each core computes its 4 heads' Q/K/V projections, the full attention for its
batch, normalization, and its partial output projection; the host sums the two
partial outputs per batch (and adds the output bias bo).

Layout: scores are computed transposed (S^T[k, q], k on partitions); the
relative-position bias becomes per-(k-tile, q-chunk) Toeplitz blocks
(out-of-band blocks fold into the exp's per-partition bias; in-band blocks
multiply exp(scores) by precomputed exp(bias) blocks on DVE).  The AV matmul
uses V plus a ones column as the stationary operand, producing ctx^T[d, q]
and the softmax denominator in one accumulation chain.  The O-projection
stacks two heads per matmul (contraction 128).

The whole kernel is one software pipeline: body i issues scores(i) [PE],
exp(i-1) [ACT], bias-mul(i-1) [DVE], AV(i-2) [PE].  The hp1 K/Q projections
and the V projection run as pseudo-units interleaved into the first bodies,
so the scalar engine's ~145us of exp work starts ~12us into the kernel.
Normalization and the O-projection are deferred into later bodies, 1-2 ops
per body.  DMA issue instructions cost ~0.6us of the issuing engine's time,
so nothing DMA-related sits on the scalar (ACT) queue once exp begins.
"""

import sys
import types
from collections import defaultdict

import numpy as np

B = 4
S = 2048
D = 512
NHEAD = 8
DK = 64
NCORES = 8
MAX_REL = 255
NKT = S // 128   # 16 k-tiles
NU = S // 512    # 4 q-units


def _install_axon_hooks():
    """Provide antenv.axon_hooks (missing in this image) so bass_utils'
    trace path can be used; harmless when tracing is off."""
    try:
        import antenv
    except ImportError:
        return
    try:
        from antenv.axon_hooks import get_axon_ntff_profile_hook  # noqa: F401
        return
    except ImportError:
        pass
    hook = None
    try:
        from trn_agent_boot.trn_boot import _ntff_profile_via_ctypes
        hook = _ntff_profile_via_ctypes("/opt/axon/libaxon_pjrt.so")
    except Exception:
        hook = None
    m = types.ModuleType("antenv.axon_hooks")
    m.get_axon_ntff_profile_hook = lambda: hook
    m.set_axon_ntff_profile_hook = lambda h: None
    sys.modules["antenv.axon_hooks"] = m
    antenv.axon_hooks = m


_install_axon_hooks()

import concourse.bass as bass  # noqa: E402
import concourse.bacc as bacc  # noqa: E402
import concourse.mybir as mybir  # noqa: E402
from concourse import tile  # noqa: E402
from concourse.bass_utils import run_bass_kernel_spmd  # noqa: E402
from concourse.vector_clock import ScopedClock as _ScopedClock  # noqa: E402

f32 = mybir.dt.float32
f32r = mybir.dt.float32r
bf16 = mybir.dt.bfloat16
f16 = mybir.dt.float16
AF = mybir.ActivationFunctionType


def _patched_drain_and_barrier(self, tick_clock, wait_clock):
    # walrus in this container rejects >2 sem waits on one instruction; emit
    # the tail-drain waits as standalone wait instructions instead.
    nc = self.nc
    dummy = mybir.InstNoOp(name="drain-wait-probe", engine=mybir.EngineType.SP)
    wait_clock.add_sem_waits(dummy, _ScopedClock({None: tick_clock.global_clock}))
    handles = {h.name: h for h in self.sems.allocated().values()}
    si = dummy.sync_info
    for w in (si.on_wait if si is not None else []):
        nc.sync.wait_ge(handles[w.ant_name], w.wait_value)
    nc.sync.drain()
    nc.all_engine_barrier()
    popped = nc._tile_sem_poison_stack.pop()
    assert popped is self._sem_poison
    nc.clear_and_free_semaphores(list(self.sems.allocated().values()))
    nc.all_engine_barrier()


tile.TileContext._drain_and_barrier = _patched_drain_and_barrier


def _delta(t, u):
    # key-tile offset minus query-chunk offset; bias entry index is
    # delta + (p - f) + 255 clipped to [0, 510]
    return 128 * t - 512 * u


def _cls(t, u):
    d = _delta(t, u)
    if d <= -384:
        return 1  # whole block clamps to table[0]
    if d >= 768:
        return 2  # whole block clamps to table[510]
    return 0      # in-band: needs the Toeplitz block


def _didx(t, u):
    return (_delta(t, u) + 256) // 128  # 0..7 for in-band blocks


def _unit_list():
    """Pipeline unit sequence: attention units for (u, hp, g, ah) with ALL
    projections interleaved as pseudo units (kq: K/Q projection chunk,
    vg: V projection pair) so there is no serial projection phase at all.
    kq `which`: 0=K hp0, 1=Q hp0, 2=K hp1, 3=Q hp1."""
    units = [("kq", 0, 0, 0), ("kq", 1, 0, 0), ("vg", 0, 0, 0)]
    after_hp0 = {0: [("vg", 1, 0, 0)],
                 1: [("kq", 0, 1, 0), ("vg", 2, 0, 0)],
                 2: [("vg", 3, 0, 0)],
                 3: [("kq", 0, 2, 0), ("vg", 4, 0, 0)],
                 4: [("vg", 5, 0, 0), ("kq", 2, 0, 0)],
                 5: [("kq", 0, 3, 0), ("vg", 6, 0, 0)],
                 6: [("vg", 7, 0, 0), ("kq", 3, 0, 0)]}
    for g in range(8):
        units.append(("att", 0, 0, (g, 0)))
        units.append(("att", 0, 0, (g, 1)))
        units.extend(after_hp0.get(g, []))
    after_hp1 = {0: [("kq", 2, 1, 0)],
                 2: [("kq", 2, 2, 0), ("kq", 1, 1, 0)],
                 4: [("kq", 2, 3, 0), ("kq", 1, 2, 0)],
                 6: [("kq", 1, 3, 0), ("kq", 3, 1, 0)],
                 7: [("kq", 3, 2, 0), ("kq", 3, 3, 0)]}
    for g in range(8):
        units.append(("att", 0, 1, (g, 0)))
        units.append(("att", 0, 1, (g, 1)))
        units.extend(after_hp1.get(g, []))
    for u in range(NU):
        for hp in range(2):
            if u == 0:
                continue
            for g in range(8):
                for ah in range(2):
                    units.append(("att", u, hp, (g, ah)))
    return units


def build_program():
    nc = bacc.Bacc()

    xqT = nc.declare_dram_parameter("xqT", [D, S], f16, isOutput=False)
    xkT = nc.declare_dram_parameter("xkT", [D, S], f16, isOutput=False)
    xvT = nc.declare_dram_parameter("xvT", [D, S], f16, isOutput=False)
    wq = nc.declare_dram_parameter("wq", [128, 4, 256], f16, isOutput=False)
    wk = nc.declare_dram_parameter("wk", [128, 4, 256], f16, isOutput=False)
    wv = nc.declare_dram_parameter("wv", [128, 4, 256], f16, isOutput=False)
    wo = nc.declare_dram_parameter("wo", [128, 2, 512], f16, isOutput=False)
    ebd = nc.declare_dram_parameter("eb", [128, 4, 8, 512], f16, isOutput=False)
    cbd = nc.declare_dram_parameter("cb", [128, 4, 3], f32, isOutput=False)
    bcd = nc.declare_dram_parameter("bcp", [2, 128], f16, isOutput=False)
    outd = nc.declare_dram_parameter("out", [S, D], f32, isOutput=True)

    with tile.TileContext(nc) as tc:
        with (
            tc.tile_pool(name="sb", bufs=1) as pool,
            tc.tile_pool(name="wkk", bufs=4) as wkp,
            tc.tile_pool(name="ps", bufs=1, space="PSUM") as psp,
        ):
            # ---- persistent SBUF tiles -------------------------------------
            wq_sb = pool.tile([128, 4, 256], f16, tag="wq", name="wq_sb")
            wk_sb = pool.tile([128, 4, 256], f16, tag="wk", name="wk_sb")
            wv_sb = pool.tile([128, 4, 256], f16, tag="wv", name="wv_sb")
            wo_sb = pool.tile([128, 2, 512], f16, tag="wo", name="wo_sb")
            eb_sb = pool.tile([128, 4, 8, 512], f16, tag="eb", name="eb_sb")
            cb_sb = pool.tile([128, 4, 3], f32, tag="cb", name="cb_sb")
            qt_sb = pool.tile([128, 2, S], f16, tag="qt", name="qt_sb")
            kt_sb = pool.tile([128, 2, S], f16, tag="kt", name="kt_sb")
            v_sb = pool.tile([128, NKT, 4 * 65], f16, tag="v", name="v_sb")
            xq_sb = pool.tile([128, 4, S], f16, tag="xq", name="xq_sb")
            xk_sb = pool.tile([128, 4, S], f16, tag="xk", name="xk_sb")
            xv_sb = pool.tile([128, 4, S], f16, tag="xv", name="xv_sb")
            bc1 = pool.tile([2, 128], f16, tag="bc1", name="bc1")
            warm = pool.tile([128, 16], f32, tag="warm", name="warm")

            # ---- input DMAs ------------------------------------------------
            nc.sync.dma_start(wk_sb[:], wk[:])
            nc.scalar.dma_start(wq_sb[:], wq[:])
            nc.gpsimd.dma_start(cb_sb[:], cbd[:])
            nc.gpsimd.dma_start(bc1[:], bcd[:])
            nc.gpsimd.dma_start(wv_sb[:], wv[:])
            for sc in range(4):
                cols = slice(sc * 512, (sc + 1) * 512)
                nc.sync.dma_start(
                    xk_sb[:, :, cols],
                    xkT[:, cols].rearrange("(c p) x -> p c x", p=128))
                nc.scalar.dma_start(
                    xq_sb[:, :, cols],
                    xqT[:, cols].rearrange("(c p) x -> p c x", p=128))
                nc.gpsimd.dma_start(
                    xv_sb[:, :, cols],
                    xvT[:, cols].rearrange("(c p) x -> p c x", p=128))
            # eb is big (4.2MB): hp0's half is needed by ~body 3, hp1's much
            # later; keep both on the otherwise-idle gpsimd queue.
            nc.gpsimd.dma_start(eb_sb[:, 0:2, :, :], ebd[:, 0:2, :, :])
            nc.gpsimd.dma_start(wo_sb[:], wo[:])
            nc.gpsimd.dma_start(eb_sb[:, 2:4, :, :], ebd[:, 2:4, :, :])

            # preload the exp table while DMAs stream in
            nc.vector.memset(warm[:], 0.0)
            nc.scalar.activation(warm[:], warm[:], AF.Exp, bias=0.0, scale=1.0)
            nc.vector.memset(
                v_sb.rearrange("p s (h x) -> p s h x", x=65)[:, :, :, 64:65],
                1.0)

            # ---- the pipeline ----------------------------------------------
            UNITS = _unit_list()
            NUNITS = len(UNITS)

            state = {}
            ctxp_tiles = {}
            ctxf_tiles = {}
            cx_tiles = {}
            navs = {}
            attpos = defaultdict(int)
            sched = defaultdict(list)

            def defer(body, fn):
                sched[body].append(fn)

            def issue_S(i):
                kind, a1, a2, a3 = UNITS[i]
                sct = psp.tile([128, 1024], f32, tag="sct", bufs=2,
                               name=f"sct{i}")
                if kind == "att":
                    u, hp, (g, ah) = a1, a2, a3
                    for ti in range(2):
                        t = 2 * g + ti
                        nc.tensor.matmul(
                            sct[:, ti * 512:(ti + 1) * 512],
                            lhsT=kt_sb[ah * 64:(ah + 1) * 64, hp,
                                       t * 128:(t + 1) * 128],
                            rhs=qt_sb[ah * 64:(ah + 1) * 64, hp,
                                      u * 512:(u + 1) * 512],
                            start=True, stop=True,
                            tile_position=(ah * 64, 0),
                        )
                elif kind == "vg":
                    g = a1
                    for s2 in range(2):
                        st = 2 * g + s2
                        for ct in range(4):
                            nc.tensor.matmul(
                                sct[:, s2 * 256:(s2 + 1) * 256],
                                lhsT=xv_sb[:, ct, st * 128:(st + 1) * 128],
                                rhs=wv_sb[:, ct, :],
                                start=(ct == 0), stop=(ct == 3),
                            )
                else:  # kq: K/Q projection, one sc chunk
                    which, sc = a1, a2
                    w_t = wq_sb if which % 2 else wk_sb
                    x_t = xq_sb if which % 2 else xk_sb
                    hp = which // 2
                    for ct in range(4):
                        nc.tensor.matmul(
                            sct[:, 0:512],
                            lhsT=w_t[:, ct, hp * 128:(hp + 1) * 128],
                            rhs=x_t[:, ct, sc * 512:(sc + 1) * 512],
                            start=(ct == 0), stop=(ct == 3),
                        )
                state[i] = {"sct": sct}

            def issue_EM(i):
                kind, a1, a2, a3 = UNITS[i]
                sct = state[i]["sct"]
                if kind == "att":
                    u, hp, (g, ah) = a1, a2, a3
                    lh = 2 * hp + ah
                    cls = _cls(2 * g, u)
                    pt = wkp.tile([128, 1024], f16, tag="pt", bufs=4,
                                  name=f"pt{i}")
                    nc.scalar.activation(
                        pt[:], sct[:], AF.Exp,
                        bias=cb_sb[:, lh, cls:cls + 1], scale=1.0,
                    )
                    if cls == 0:
                        src = wkp.tile([128, 1024], f16, tag="srcx", bufs=4,
                                       name=f"src{i}")
                        for ti in range(2):
                            nc.vector.tensor_mul(
                                src[:, ti * 512:(ti + 1) * 512],
                                pt[:, ti * 512:(ti + 1) * 512],
                                eb_sb[:, lh, _didx(2 * g + ti, u), :],
                            )
                    else:
                        src = pt
                    state[i]["src"] = src
                elif kind == "vg":
                    g = a1
                    nc.vector.tensor_copy(
                        v_sb[:, 2 * g:2 * g + 2, :]
                        .rearrange("p s (h x) -> p s h x", x=65)[:, :, :, 0:64],
                        sct[:, 0:512].rearrange("p (s h x) -> p s h x",
                                                s=2, h=4),
                    )
                else:
                    which, sc = a1, a2
                    dst = qt_sb if which % 2 else kt_sb
                    nc.vector.tensor_copy(
                        dst[:, which // 2, sc * 512:(sc + 1) * 512],
                        sct[:, 0:512])

            def issue_A(i):
                _, u, hp, (g, ah) = UNITS[i]
                lh = 2 * hp + ah
                key = (u, hp, ah)
                if key not in ctxp_tiles:
                    ctxp_tiles[key] = psp.tile(
                        [65, 512], f32, tag="ctx", bufs=2,
                        name=f"ctx{u}{hp}{ah}")
                    navs[key] = 0
                src = state[i]["src"]
                for ti in range(2):
                    t = 2 * g + ti
                    navs[key] += 1
                    nc.tensor.matmul(
                        ctxp_tiles[key][:],
                        lhsT=v_sb[:, t, lh * 65:(lh + 1) * 65],
                        rhs=src[:, ti * 512:(ti + 1) * 512],
                        start=(navs[key] == 1), stop=(navs[key] == 16),
                    )

            def sched_norm(i_last, u, hp):
                # A(i_last) issues at body i_last+2; normalization follows.
                # The very last block gets a dense schedule (its deferred
                # bodies all fire back-to-back in the pipeline tail, and the
                # ACT queue is idle there, so its issue slots are free).
                b0 = i_last + 3
                last = (u == NU - 1 and hp == 1)

                def cp(ah):
                    def fn():
                        ctxf = wkp.tile([65, 512], f32, tag="cxf", bufs=4,
                                        name=f"cxf{u}{hp}{ah}")
                        nc.vector.tensor_copy(ctxf[:],
                                              ctxp_tiles[(u, hp, ah)][:])
                        ctxf_tiles[(u, hp, ah)] = ctxf
                    return fn

                defer(b0, cp(0))
                defer(b0 if last else b0 + 1, cp(1))

                lpt = {}

                def lp_dma():
                    lp = wkp.tile([2, 512], f32, tag="lp", bufs=2,
                                  name=f"lp{u}{hp}")
                    q0 = nc.sync
                    q1 = nc.scalar if last else nc.sync
                    q0.dma_start(lp[0:1, :],
                                 ctxf_tiles[(u, hp, 0)][64:65, :])
                    q1.dma_start(lp[1:2, :],
                                 ctxf_tiles[(u, hp, 1)][64:65, :])
                    lpt["lp"] = lp
                defer(b0 + 1, lp_dma)

                def recip():
                    linv = wkp.tile([2, 512], f32, tag="linv", bufs=2,
                                    name=f"linv{u}{hp}")
                    linvb = wkp.tile([2, 512], f16, tag="linvb", bufs=2,
                                     name=f"linvb{u}{hp}")
                    nc.vector.reciprocal_approx_fast(linv[:], lpt["lp"][:])
                    nc.vector.tensor_scalar_mul(linvb[:], linv[:], 16.0)
                    lpt["linvb"] = linvb
                defer(b0 + 2 if last else b0 + 3, recip)

                def bc_mm():
                    bc = psp.tile([128, 512], f32, tag="bc", bufs=1,
                                  name=f"bc{u}{hp}")
                    nc.tensor.matmul(bc[:], lhsT=bc1[:], rhs=lpt["linvb"][:],
                                     start=True, stop=True)
                    lpt["bc"] = bc
                defer(b0 + 3 if last else b0 + 4, bc_mm)

                def cx_mul(ah):
                    def fn():
                        key = (u, hp)
                        if key not in cx_tiles:
                            cx_tiles[key] = wkp.tile(
                                [128, 512], f16, tag="cx", bufs=4,
                                name=f"cx{u}{hp}")
                        nc.vector.tensor_mul(
                            cx_tiles[key][ah * 64:(ah + 1) * 64, :],
                            lpt["bc"][ah * 64:(ah + 1) * 64, :],
                            ctxf_tiles[(u, hp, ah)][0:64, :],
                        )
                    return fn
                cb_ = b0 + 4 if last else b0 + 5
                defer(cb_, cx_mul(0))
                defer(cb_, cx_mul(1))

                if hp == 1:
                    pb = b0 + 5 if last else b0 + 7
                    step = 2 if last else 3
                    for qs in range(4):
                        pot = {}

                        def po_mm(qs, pot):
                            def fn():
                                pot["po"] = psp.tile(
                                    [128, 512], f32, tag="po", bufs=1,
                                    name=f"po{u}{qs}")
                                for hp2 in range(2):
                                    nc.tensor.matmul(
                                        pot["po"][:],
                                        lhsT=cx_tiles[(u, hp2)][
                                            :, qs * 128:(qs + 1) * 128],
                                        rhs=wo_sb[:, hp2, :],
                                        start=(hp2 == 0), stop=(hp2 == 1),
                                    )
                            return fn

                        def po_out(qs, pot):
                            def fn():
                                ob = wkp.tile([128, 512], f32, tag="ob",
                                              bufs=2, name=f"ob{u}{qs}")
                                nc.vector.tensor_copy(ob[:], pot["po"][:])
                                qo = nc.gpsimd if (last and qs % 2) else nc.sync
                                qo.dma_start(
                                    outd[u * 512 + qs * 128:
                                         u * 512 + (qs + 1) * 128, :],
                                    ob[:],
                                )
                            return fn

                        defer(pb + qs * step, po_mm(qs, pot))
                        defer(pb + qs * step + 1, po_out(qs, pot))

            # ---- pipeline loop ---------------------------------------------
            for body in range(NUNITS + 64):
                if body < NUNITS:
                    issue_S(body)
                    kind, u, hp, a3 = UNITS[body]
                    if kind == "att":
                        g, ah = a3
                        p = attpos[(u, hp)]
                        attpos[(u, hp)] += 1
                        skew = 4 if p < 2 else 2
                        defer(body + skew, (lambda i: lambda: issue_A(i))(body))
                        if g == 7 and ah == 1:
                            sched_norm(body, u, hp)
                if 0 <= body - 1 < NUNITS:
                    issue_EM(body - 1)
                for fn in sched.pop(body, []):
                    fn()

    nc.compile()
    return nc


_PROGRAM = None


def _get_program():
    global _PROGRAM
    if _PROGRAM is None:
        _PROGRAM = build_program()
    return _PROGRAM


# index table for the in-band Toeplitz bias blocks, shared across heads
_IDX = None


def _idx_table():
    global _IDX
    if _IDX is None:
        p = np.arange(128)[:, None]
        f = np.arange(512)[None, :]
        blocks = []
        for didx in range(8):
            delta = didx * 128 - 256
            blocks.append(np.clip(delta + p - f + 255, 0, 510))
        _IDX = np.stack(blocks, axis=0)  # [8, 128, 512]
    return _IDX


def kernel(**inputs):
    query = np.asarray(inputs["query"], dtype=np.float32)
    key = np.asarray(inputs["key"], dtype=np.float32)
    value = np.asarray(inputs["value"], dtype=np.float32)
    mask = np.asarray(inputs["mask"])
    Wq = np.asarray(inputs["Wq"], dtype=np.float32)
    Wk = np.asarray(inputs["Wk"], dtype=np.float32)
    Wv = np.asarray(inputs["Wv"], dtype=np.float32)
    Wo = np.asarray(inputs["Wo"], dtype=np.float32)
    bo = np.asarray(inputs["bo"], dtype=np.float32)
    rel_bias = np.asarray(inputs["rel_bias"], dtype=np.float32)

    if not np.all(mask != 0):
        raise NotImplementedError("kernel assumes an all-ones attention mask")

    nc = _get_program()
    idx = _idx_table()
    scale = np.float32(1.0 / np.sqrt(DK))

    in_maps = []
    for c in range(NCORES):
        b = c // 2
        hbase = (c % 2) * 4
        rows = slice(hbase * 64, (hbase + 4) * 64)

        wq_arr = np.ascontiguousarray(
            (Wq[rows, :] * scale).T.reshape(4, 128, 256).swapaxes(0, 1))
        wk_arr = np.ascontiguousarray(
            Wk[rows, :].T.reshape(4, 128, 256).swapaxes(0, 1))
        wv_arr = np.ascontiguousarray(
            Wv[rows, :].T.reshape(4, 128, 256).swapaxes(0, 1))

        wo_arr = np.empty((128, 2, 512), dtype=np.float32)
        eb_arr = np.empty((128, 4, 8, 512), dtype=np.float16)
        cb_arr = np.zeros((128, 4, 3), dtype=np.float32)
        for lh in range(4):
            g = hbase + lh
            wo_arr[(lh % 2) * 64:(lh % 2) * 64 + 64, lh // 2, :] = \
                Wo[:, g * 64:(g + 1) * 64].T * (1.0 / 16.0)
            tbl = rel_bias[g]
            eb_arr[:, lh, :, :] = np.exp(tbl)[idx].transpose(1, 0, 2)
            cb_arr[:, lh, 1] = tbl[0]
            cb_arr[:, lh, 2] = tbl[510]

        bcp = np.zeros((2, 128), dtype=np.float16)
        bcp[0, 0:64] = 1.0
        bcp[1, 64:128] = 1.0

        bf = np.float16
        in_maps.append({
            "bcp": bcp,
            "xqT": np.ascontiguousarray(query[b].T).astype(bf),
            "xkT": np.ascontiguousarray(key[b].T).astype(bf),
            "xvT": np.ascontiguousarray(value[b].T).astype(bf),
            "wq": wq_arr.astype(bf), "wk": wk_arr.astype(bf),
            "wv": wv_arr.astype(bf), "wo": wo_arr.astype(bf),
            "eb": eb_arr, "cb": cb_arr,
        })

    res = run_bass_kernel_spmd(nc, in_maps, list(range(NCORES)), trace=False)

    out = np.zeros((B, S, D), dtype=np.float32)
    for c in range(NCORES):
        out[c // 2] += res.results[c]["out"]
    out += bo[None, None, :]
    return out
